# revision 28
# baseline (speedup 1.0000x reference)
"""Trainium2 Bass kernel for AdaptiveNoisingModule (retrieval_knn).

Computation (matches reference.py):
  x[N,D] queries from features; cdist to memory bank [M,D]; top-9 nearest;
  influence = mean_k |x - nn_k| * w; per-row normalize; distance signal from
  mean top-9 distance, normalized by GLOBAL mean/std (the only collective);
  noise_std = 0.01 + 0.49*sigmoid(infl_norm + dw*dist_norm);
  outputs: noised = x + eps*noise_std (as [B,C,H,W]), influence_map, noise_std_map.

Sharding: queries (N = B*H*W) across 8 cores, one batch image per core
(784 queries each). Memory bank replicated. One tiny AllReduce (2 floats)
for the global distance-signal stats.

Device algorithm per core:
  Phase A: stream bank once; fp32r matmuls compute selection score
    s = -||m||^2 + x.(2m)  (monotone in -d2 per row). Per 1024-col chunk,
    DVE max8 + max_index give per-chunk top-8 candidates (2 full passes
    over M total). Top-9 of the union of per-chunk top-8s == exact top-9
    unless >=9 of the true top-9 land in one chunk (P ~ 2e-10 per row).
  Merge: max8/match_replace/max8/max_index on the 128-wide candidate list,
    then an iota-select recovers each winner's global index.
  Phase B: indirect-DMA gather of the 9 neighbor rows; diff/abs/reduce for
    influence; exact d2 = sum((x-nn)^2) recomputed for distances; row stats.
  Phase C: AllReduce of [sum(ds'), sum(ds'^2)]; sigmoid + noise application.
"""

import os
import sys

import numpy as np

if "/opt/trn_rl_repo" not in sys.path:
    sys.path.insert(0, "/opt/trn_rl_repo")

# ---------------------------------------------------------------- constants
B, C, H, W = 8, 1024, 28, 28
M_FULL, D_FULL = 16384, 1024
N_CORES = 8
N_LOC_FULL = (B * H * W) // N_CORES  # 784
KNN = 9
MIN_STD, MAX_STD = 0.01, 0.5
EPS_NORM = 1e-8
# numerical-stability shift for the global ds sums (any value near E[ds] works)
C_SHIFT = 42.3


def _row_tiles(n_loc):
    tiles = []
    off = 0
    while off < n_loc:
        p = min(128, n_loc - off)
        tiles.append((off, p))
        off += p
    return tiles


def build_program(n_loc=N_LOC_FULL, M=M_FULL, D=D_FULL, n_cores=N_CORES,
                  n_chunks=16, diff_on_gpsimd=False, legalize=True):
    """Builds the per-core Bass program (SPMD: same program on all cores)."""
    import concourse.bass as bass
    import concourse.mybir as mybir
    import concourse.tile as tile
    from concourse.bass import IndirectOffsetOnAxis

    f32 = mybir.dt.float32
    f32r = mybir.dt.float32r
    u32 = mybir.dt.uint32
    AX = mybir.AxisListType.X
    OP = mybir.AluOpType
    AF = mybir.ActivationFunctionType

    assert D % 128 == 0
    KT = D // 128
    assert M % n_chunks == 0
    CH = M // n_chunks            # chunk width for selection
    MB = min(512, CH)             # matmul free-dim block
    assert CH % MB == 0
    MBC = CH // MB
    CW = 8 * n_chunks             # candidate width
    tiles = _row_tiles(n_loc)
    T = len(tiles)
    N_GLOB = n_loc * n_cores
    knn = min(KNN, M)

    nc = bass.Bass(num_devices=n_cores)

    xT = nc.declare_dram_parameter("xT", [D, n_loc], f32r, False)
    xr = nc.declare_dram_parameter("xrows", [n_loc, D], f32, False)
    bankT = nc.declare_dram_parameter("bankT2", [D, M], f32r, False)
    bank = nc.declare_dram_parameter("bank", [M, D], f32, False)
    bsqn = nc.declare_dram_parameter("bsqn", [1, M], f32r, False)
    epsd = nc.declare_dram_parameter("eps", [n_loc, D], f32, False)
    w9d = nc.declare_dram_parameter("w9", [128, D], f32, False)
    iotad = nc.declare_dram_parameter("iota", [128, CW], f32, False)
    cbased = nc.declare_dram_parameter("cbase", [128, CW], f32, False)
    dwd = nc.declare_dram_parameter("dw", [128, 1], f32, False)
    outN = nc.declare_dram_parameter("out_noised", [n_loc, D], f32, True)
    outM = nc.declare_dram_parameter("out_maps", [2, n_loc], f32, True)
    outD = nc.declare_dram_parameter("out_dbg", [128, 128], f32, True)

    with tile.TileContext(nc) as tc:
        with (
            tc.tile_pool(name="cpool", bufs=1) as cpool,
            tc.tile_pool(name="rhs", bufs=KT * MBC) as rhs_pool,
            tc.tile_pool(name="bsq", bufs=2) as bsq_pool,
            tc.tile_pool(name="psum", bufs=7, space="PSUM") as psum_pool,
            tc.tile_pool(name="psmall", bufs=1, space="PSUM") as psmall_pool,
            tc.tile_pool(name="slab", bufs=2) as slab_pool,
            tc.tile_pool(name="gpool", bufs=1) as g_pool,
            tc.tile_pool(name="xpool", bufs=2) as x_pool,
            tc.tile_pool(name="epool", bufs=2) as eps_pool,
            tc.tile_pool(name="scr", bufs=2) as scr_pool,
            tc.tile_pool(name="sig", bufs=2) as sig_pool,
            tc.tile_pool(name="sm", bufs=3) as sm,
            tc.tile_pool(name="dram", bufs=1, space="DRAM") as dram_pool,
        ):
            # ---------------- persistent tiles
            xT_s = cpool.tile([128, KT * n_loc], f32r)
            w9_s = cpool.tile([128, D], f32)
            iota_s = cpool.tile([128, CW], f32)
            cbase_s = cpool.tile([128, CW], f32)
            dw_s = cpool.tile([128, 1], f32)
            ones_r = cpool.tile([1, 128], f32r)     # row of ones (seed lhsT)
            ones_c = cpool.tile([128, 1], f32)     # column of ones (reduce lhsT)
            ones_rf = cpool.tile([1, 128], f32)    # f32 ones row (broadcast lhsT)
            cval_s = cpool.tile([128, T * CW], f32)
            cidx_s = cpool.tile([128, T * CW], u32)
            nidxf_s = cpool.tile([128, T * knn], f32)
            nidx_s = cpool.tile([128, T * knn], u32)
            d2col_s = cpool.tile([128, T * knn], f32)
            infl_s = cpool.tile([128, T * D], f32)
            dstats = cpool.tile([128, 2 * T], f32)   # cols [0,T)=ds', [T,2T)=ds'^2
            meanb_s = cpool.tile([128, T], f32)
            negmb_s = cpool.tile([128, T], f32)
            invb_s = cpool.tile([128, T], f32)

            cc_in = dram_pool.tile([1, 8], f32)
            cc_out = dram_pool.tile([1, 8], f32)

            nc.sync.dma_start(
                out=xT_s[:].rearrange("p (k n) -> p k n", k=KT),
                in_=xT[:, :].rearrange("(k p) n -> p k n", p=128))
            nc.sync.dma_start(out=w9_s[:], in_=w9d[:])
            nc.sync.dma_start(out=iota_s[:], in_=iotad[:])
            nc.sync.dma_start(out=cbase_s[:], in_=cbased[:])
            nc.sync.dma_start(out=dw_s[:], in_=dwd[:])
            nc.vector.memset(ones_r[:].bitcast(f32), 1.0)
            nc.vector.memset(ones_rf[:], 1.0)
            nc.vector.memset(ones_c[:], 1.0)
            nc.vector.memset(dstats[:], 0.0)
            nc.vector.memset(nidxf_s[:], 0.0)
            nc.vector.memset(meanb_s[:], 0.0)
            nc.vector.memset(negmb_s[:], 0.0)
            nc.vector.memset(invb_s[:], 0.0)

            # Wait-absorbers: self-loading 4-byte matmuls can carry only one
            # sync wait, so tiny PE transposes soak up semaphore waits first.
            def absorb(dep_ap, out_ap):
                nc.tensor.transpose(out_ap, dep_ap, ones_rf[:1, 0:1])

            abs0 = psmall_pool.tile([1, 8], f32, tag="pss")
            absorb(ones_rf[:1, 0:1], abs0[:1, 0:1])  # DVE memset sem
            absorb(ones_r[:1, 0:1].bitcast(f32), abs0[:1, 1:2])
            absorb(xT_s[:1, 0:1].bitcast(f32), abs0[:1, 2:3])  # xT DMA sem
            jnk = psmall_pool.tile([1, 16], f32, tag="pss")
            jrot = [0]
            # first toucher of the jnk slot (takes the slot-free PE wait)
            nc.tensor.transpose(jnk[:1, 15:16], ones_rf[:1, 0:1],
                                ones_rf[:1, 0:1])

            def jslot():
                j = jrot[0] % 16
                jrot[0] += 1
                return j

            # ================= PHASE A: matmuls + per-chunk top-8 ============
            for c in range(n_chunks):
                bsq_t = bsq_pool.tile([1, CH], f32r)
                nc.gpsimd.dma_start(out=bsq_t[:], in_=bsqn[:, c * CH:(c + 1) * CH])
                rhs_tiles = []
                for k in range(KT):
                    row = []
                    for mb in range(MBC):
                        r = rhs_pool.tile([128, MB], f32r, tag="rhs")
                        mcol = c * CH + mb * MB
                        nc.gpsimd.dma_start(
                            out=r[:], in_=bankT[k * 128:(k + 1) * 128,
                                               mcol:mcol + MB])
                        row.append(r)
                    rhs_tiles.append(row)
                for t, (toff, P) in enumerate(tiles):
                    slab = slab_pool.tile([128, CH], f32)
                    for mb in range(MBC):
                        ps = psum_pool.tile([128, MB], f32)
                        if t == 0 and mb == 0:
                            # absorb the bsq DMA wait before any seed needs it
                            nc.tensor.transpose(
                                jnk[:1, jslot():][:, :1],
                                bsq_t[:1, 0:1].bitcast(f32), ones_rf[:1, 0:1])
                        # seed with -||m||^2 (K=1 matmul), then accumulate x.(2m)
                        nc.tensor.matmul(
                            ps[:P], lhsT=ones_r[:1, :P],
                            rhs=bsq_t[:1, mb * MB:(mb + 1) * MB],
                            start=True, stop=False)
                        for k in range(KT):
                            nc.tensor.matmul(
                                ps[:P],
                                lhsT=xT_s[:, k * n_loc + toff:
                                          k * n_loc + toff + P],
                                rhs=rhs_tiles[k][mb][:],
                                start=False, stop=(k == KT - 1))
                        nc.scalar.copy(out=slab[:P, mb * MB:(mb + 1) * MB],
                                       in_=ps[:P])
                        # post-evac PE toucher: folds the ACT tick into PE's
                        # clock so the next psum reuse needs only a PE wait
                        nc.tensor.transpose(
                            jnk[:1, jslot():][:, :1],
                            slab[:1, mb * MB:mb * MB + 1], ones_rf[:1, 0:1])
                    cv8 = cval_s[:P, t * CW + c * 8: t * CW + c * 8 + 8]
                    nc.vector.max(out=cv8, in_=slab[:P])
                    nc.vector.max_index(
                        cidx_s[:P, t * CW + c * 8: t * CW + c * 8 + 8],
                        cv8, slab[:P])
                    atch = sm.tile([1, 1], f32, tag="atch")
                    nc.scalar.copy(
                        out=atch[:1],
                        in_=cidx_s[:1, t * CW + c * 8:
                                   t * CW + c * 8 + 1].bitcast(f32))

            # ================= MERGE: exact top-9 from candidates ============
            for t, (toff, P) in enumerate(tiles):
                cv = cval_s[:P, t * CW:(t + 1) * CW]
                cift = sm.tile([128, CW], f32, tag="cif")
                cif = cift[:P]
                nc.vector.tensor_copy(out=cif, in_=cidx_s[:P, t * CW:(t + 1) * CW])
                nc.vector.tensor_add(out=cif, in0=cif, in1=cbase_s[:P])
                t8 = sm.tile([128, 8], f32, tag="t8")
                nc.vector.max(out=t8[:P], in_=cv)
                msk = sm.tile([128, CW], f32, tag="msk")
                nc.vector.match_replace(out=msk[:P], in_to_replace=t8[:P],
                                        in_values=cv, imm_value=-3e38)
                t9 = sm.tile([128, 8], f32, tag="t9")
                nc.vector.max(out=t9[:P], in_=msk[:P])
                p8 = sm.tile([128, 8], u32, tag="p8")
                nc.vector.max_index(p8[:P], t8[:P], cv)
                p9 = sm.tile([128, 8], u32, tag="p9")
                nc.vector.max_index(p9[:P], t9[:P], msk[:P])
                posf = sm.tile([128, 16], f32, tag="posf")
                nc.vector.tensor_copy(out=posf[:P, 0:8], in_=p8[:P])
                nc.vector.tensor_copy(out=posf[:P, 8:9], in_=p9[:P, 0:1])
                for j in range(knn):
                    sel = sm.tile([128, CW], f32, tag="sel")
                    nc.vector.scalar_tensor_tensor(
                        out=sel[:P], in0=iota_s[:P], scalar=posf[:P, j:j + 1],
                        in1=cif, op0=OP.is_equal, op1=OP.mult)
                    nc.vector.reduce_sum(
                        out=nidxf_s[:P, t * knn + j:t * knn + j + 1],
                        in_=sel[:P], axis=AX)
                nc.vector.tensor_copy(out=nidx_s[:P, t * knn:(t + 1) * knn],
                                      in_=nidxf_s[:P, t * knn:(t + 1) * knn])

                # ============ PHASE B1: gather, influence, distances =========
                xt = x_pool.tile([128, D], f32, tag="xt")
                nc.gpsimd.dma_start(out=xt[:P], in_=xr[toff:toff + P, :])
                g = g_pool.tile([128, knn * D], f32)
                for j in range(knn):
                    nc.gpsimd.indirect_dma_start(
                        out=g[:P, j * D:(j + 1) * D], out_offset=None,
                        in_=bank[:, :],
                        in_offset=IndirectOffsetOnAxis(
                            ap=nidx_s[:P, t * knn + j:t * knn + j + 1],
                            axis=0))
                atg = sm.tile([1, 1], f32, tag="atch")
                nc.scalar.copy(out=atg[:1], in_=g[:1, 0:1])
                # DVE touchers: fold the gather/xt DMA ticks and the previous
                # tile's ACT square tick into DVE's clock before the subtract
                vt = sm.tile([1, 4], f32, tag="vtb")
                nc.vector.tensor_copy(out=vt[:1, 0:1], in_=g[:1, 0:1])
                nc.vector.tensor_copy(out=vt[:1, 1:2], in_=xt[:1, 0:1])
                if t > 0:
                    nc.vector.tensor_copy(
                        out=vt[:1, 2:3],
                        in_=d2col_s[:1, t * knn - 1:t * knn])
                # diff in place: g[:, j, :] -= x
                g3 = g[:P].rearrange("p (j d) -> p j d", j=knn)
                xb = xt[:P].unsqueeze(1).to_broadcast([P, knn, D])
                if diff_on_gpsimd:
                    nc.gpsimd.tensor_sub(out=g3, in0=g3, in1=xb)
                else:
                    nc.vector.tensor_sub(out=g3, in0=g3, in1=xb)
                # exact d2 per neighbor: ACT square + accumulate
                for j in range(knn):
                    scrj = scr_pool.tile([128, D], f32, tag="scr")
                    nc.scalar.activation(
                        out=scrj[:P], in_=g[:P, j * D:(j + 1) * D],
                        func=AF.Square,
                        accum_out=d2col_s[:P, t * knn + j:t * knn + j + 1])
                # influence = (sum_j |diff|) * (w/9)
                infl = infl_s[:P, t * D:(t + 1) * D]
                nc.vector.tensor_reduce(
                    out=infl, in_=g[:P].rearrange("p (j d) -> p d j", j=knn),
                    axis=AX, op=OP.add, apply_absolute_value=True)
                nc.vector.tensor_mul(out=infl, in0=infl, in1=w9_s[:P])
                # row mean / unbiased std of influence
                s1 = sm.tile([128, 1], f32, tag="s1")
                scr1 = scr_pool.tile([128, D], f32, tag="scr")
                nc.scalar.activation(out=scr1[:P], in_=infl, func=AF.Copy,
                                     accum_out=s1[:P])
                nc.vector.tensor_scalar_mul(meanb_s[:P, t:t + 1], s1[:P], 1.0 / D)
                nc.vector.tensor_scalar_mul(negmb_s[:P, t:t + 1], s1[:P], -1.0 / D)
                # influence_map output = row mean
                nc.gpsimd.dma_start(
                    out=outM[0:1, toff:toff + P].transpose([1, 0]),
                    in_=meanb_s[:P, t:t + 1])
                s2 = sm.tile([128, 1], f32, tag="s2")
                scr2 = scr_pool.tile([128, D], f32, tag="scr")
                nc.scalar.activation(out=scr2[:P], in_=infl, func=AF.Square,
                                     bias=negmb_s[:P, t:t + 1], scale=1.0,
                                     accum_out=s2[:P])
                sd = sm.tile([128, 1], f32, tag="sd")
                nc.vector.tensor_scalar_mul(sd[:P], s2[:P], 1.0 / (D - 1))
                nc.scalar.sqrt(sd[:P], sd[:P])
                nc.vector.tensor_scalar_add(sd[:P], sd[:P], EPS_NORM)
                nc.vector.reciprocal(invb_s[:P, t:t + 1], sd[:P])
                # ds' = mean_k sqrt(clip(d2)) - C_SHIFT
                dc = d2col_s[:P, t * knn:(t + 1) * knn]
                nc.vector.tensor_scalar_max(dc, dc, 1e-12)
                dsq = sm.tile([128, knn], f32, tag="dsq")
                nc.scalar.sqrt(dsq[:P], dc)
                dtmp = sm.tile([128, 1], f32, tag="dtmp")
                nc.vector.reduce_sum(out=dtmp[:P], in_=dsq[:P], axis=AX)
                nc.vector.tensor_scalar(
                    out=dstats[:P, t:t + 1], in0=dtmp[:P],
                    scalar1=1.0 / knn, scalar2=-C_SHIFT,
                    op0=OP.mult, op1=OP.add)

            # ================= PHASE C: global ds stats (AllReduce) ==========
            nc.vector.tensor_mul(out=dstats[:, T:2 * T], in0=dstats[:, 0:T],
                                 in1=dstats[:, 0:T])
            pss = psmall_pool.tile([1, 2 * T], f32, tag="pss")
            nc.tensor.transpose(pss[:1, 0:1], ones_rf[:1, 0:1],
                                ones_rf[:1, 0:1])
            nc.tensor.matmul(pss[:1], lhsT=ones_c[:, :1], rhs=dstats[:, :2 * T],
                             start=True, stop=True)
            srow = sm.tile([1, 2 * T], f32, tag="srow")
            nc.scalar.copy(out=srow[:1], in_=pss[:1])
            loc = sm.tile([1, 8], f32, tag="loc")
            nc.vector.memset(loc[:], 0.0)
            nc.vector.tensor_reduce(
                out=loc[:1, 0:2],
                in_=srow[:1].rearrange("p (a t) -> p a t", a=2), axis=AX,
                op=OP.add)
            nc.gpsimd.dma_start(out=cc_in[:], in_=loc[:1])
            nc.gpsimd.collective_compute(
                "AllReduce", OP.add,
                replica_groups=[list(range(n_cores))],
                ins=[cc_in[:].opt()], outs=[cc_out[:].opt()])
            # replicate the [1,8] AllReduce result to all 128 partitions via
            # a stride-0 DMA read, then compute the stats on every lane
            gl = sm.tile([128, 8], f32, tag="gl")
            nc.gpsimd.dma_start(out=gl[:], in_=cc_out[:].partition_broadcast(128))
            # full stats: fm' = S1/N ; var = (S2 - S1*fm')*D/(N*D-1)
            bcst = cpool.tile([128, 2], f32)
            fm = bcst[:, 0:1]
            nc.vector.tensor_scalar_mul(fm, gl[:, 0:1], 1.0 / N_GLOB)
            t1v = sm.tile([128, 1], f32, tag="t1v")
            nc.vector.tensor_mul(t1v[:], gl[:, 0:1], fm)
            var = sm.tile([128, 1], f32, tag="var")
            nc.vector.tensor_sub(var[:], gl[:, 1:2], t1v[:])
            nc.vector.tensor_scalar(
                out=var[:], in0=var[:],
                scalar1=float(D) / (N_GLOB * D - 1.0), scalar2=0.0,
                op0=OP.mult, op1=OP.max)
            nc.scalar.sqrt(var[:], var[:])
            nc.vector.tensor_scalar_add(var[:], var[:], EPS_NORM)
            nc.vector.reciprocal(bcst[:, 1:2], var[:])
            dwinv = cpool.tile([128, 1], f32)
            nc.vector.tensor_mul(dwinv[:], dw_s[:], bcst[:, 1:2])

            # -------- debug dumps
            nc.gpsimd.dma_start(out=outD[:, 0:T * knn],
                              in_=nidxf_s[:, 0:T * knn])
            nc.gpsimd.dma_start(out=outD[:, 64:64 + 2], in_=bcst[:, 0:2])
            nc.gpsimd.dma_start(out=outD[:, 66:66 + T], in_=dstats[:, 0:T])
            nc.gpsimd.dma_start(out=outD[:, 74:74 + T], in_=meanb_s[:, 0:T])
            nc.gpsimd.dma_start(out=outD[:, 82:82 + T], in_=invb_s[:, 0:T])
            nc.gpsimd.dma_start(out=outD[:, 90:90 + 8], in_=gl[:, 0:8])

            # ================= PHASE B2: noise application ===================
            for t, (toff, P) in enumerate(tiles):
                dn = sm.tile([128, 1], f32, tag="dn")
                nc.vector.tensor_sub(dn[:P], dstats[:P, t:t + 1], bcst[:P, 0:1])
                nc.vector.tensor_mul(dn[:P], dn[:P], dwinv[:P])
                bias = sm.tile([128, 1], f32, tag="bias")
                nc.vector.scalar_tensor_tensor(
                    out=bias[:P], in0=negmb_s[:P, t:t + 1],
                    scalar=invb_s[:P, t:t + 1], in1=dn[:P],
                    op0=OP.mult, op1=OP.add)
                sg = sig_pool.tile([128, D], f32, tag="sg")
                nc.scalar.activation(out=sg[:P], in_=infl_s[:P, t * D:(t + 1) * D],
                                     func=AF.Sigmoid,
                                     scale=invb_s[:P, t:t + 1], bias=bias[:P])
                smv = sm.tile([128, 1], f32, tag="smv")
                nc.vector.tensor_scalar(
                    out=sg[:P], in0=sg[:P], scalar1=MAX_STD - MIN_STD,
                    scalar2=MIN_STD, op0=OP.mult, op1=OP.add)
                scr3 = scr_pool.tile([128, D], f32, tag="scr")
                nc.scalar.activation(out=scr3[:P], in_=sg[:P], func=AF.Copy,
                                     accum_out=smv[:P])
                nc.vector.tensor_scalar_mul(smv[:P], smv[:P], 1.0 / D)
                nc.gpsimd.dma_start(
                    out=outM[1:2, toff:toff + P].transpose([1, 0]),
                    in_=smv[:P])
                ep = eps_pool.tile([128, D], f32, tag="ep")
                nc.gpsimd.dma_start(out=ep[:P], in_=epsd[toff:toff + P, :])
                xt2 = x_pool.tile([128, D], f32, tag="xt")
                nc.gpsimd.dma_start(out=xt2[:P], in_=xr[toff:toff + P, :])
                vtch = sm.tile([1, 2], f32, tag="vtch")
                nc.vector.tensor_copy(out=vtch[:1, 0:1], in_=ep[:1, 0:1])
                nc.vector.tensor_copy(out=vtch[:1, 1:2], in_=xt2[:1, 0:1])
                nc.vector.tensor_mul(out=ep[:P], in0=ep[:P], in1=sg[:P])
                nc.vector.tensor_add(out=ep[:P], in0=ep[:P], in1=xt2[:P])
                nc.gpsimd.dma_start(out=outN[toff:toff + P, :], in_=ep[:P])

    if legalize:
        _legalize_sync(nc)
    return nc


_SKIP_SYNC_SPLIT = ("InstCall", "InstUnconditionalBranch", "InstISA",
                    "InstEventSemaphore")


def _legalize_sync(nc, limit=1):
    """Walrus engine-instruction structs hold very few sync commands; split
    excess semaphore waits onto injected same-engine NOPs placed before the
    instruction (the issuing sequencer executes them in order, so semantics
    are unchanged). For DMAs, prefer keeping the DMA-queue wait inline and
    moving cross-engine waits to the NOPs."""
    import concourse.mybir as mb
    ctr = 0
    for fn in nc.m.functions:
        for bb in fn.blocks:
            out = []
            for ins in bb.instructions:
                tn = type(ins).__name__
                si = ins.sync_info
                waits = list(si.on_wait) if si is not None else []
                if tn in _SKIP_SYNC_SPLIT or len(waits) <= limit:
                    out.append(ins)
                    continue
                keep = [w for w in waits if w.ant_name.startswith("DMA")]
                move = [w for w in waits if not w.ant_name.startswith("DMA")]
                while len(keep) > limit:
                    move.append(keep.pop(0))
                while move and len(keep) < limit:
                    keep.append(move.pop())
                for w in move:
                    ctr += 1
                    nop = mb.InstNoOp(name=f"I-wsplit-{ctr}", ins=[], outs=[])
                    nop.engine = ins.engine
                    nop.sync_info = mb.SyncInfo(on_wait=[w], on_update=[])
                    out.append(nop)
                ins.sync_info = mb.SyncInfo(on_wait=keep,
                                            on_update=list(si.on_update))
                out.append(ins)
            bb.instructions = out


# ------------------------------------------------------------------- host

def make_in_maps(features, memory_bank, influence_weight, distance_weight, eps,
                 n_loc=N_LOC_FULL, M=M_FULL, D=D_FULL, n_cores=N_CORES,
                 n_chunks=16):
    CH = M // n_chunks
    CW = 8 * n_chunks
    features = np.ascontiguousarray(features, dtype=np.float32)
    memory_bank = np.ascontiguousarray(memory_bank, dtype=np.float32)
    eps = np.ascontiguousarray(eps, dtype=np.float32)
    bankT2 = np.ascontiguousarray(memory_bank.T * np.float32(2.0))
    bsqn = -(memory_bank * memory_bank).sum(axis=1, dtype=np.float32)[None, :]
    bsqn = np.ascontiguousarray(bsqn)
    w9 = np.ascontiguousarray(
        np.broadcast_to((influence_weight.astype(np.float32) / KNN)[None, :],
                        (128, D)))
    iota = np.ascontiguousarray(
        np.broadcast_to(np.arange(CW, dtype=np.float32)[None, :], (128, CW)))
    cbase = np.ascontiguousarray(
        np.broadcast_to(
            np.repeat(np.arange(n_chunks, dtype=np.float32) * CH, 8)[None, :],
            (128, CW)))
    dw = np.full((128, 1), np.float32(distance_weight.reshape(-1)[0]),
                 dtype=np.float32)
    in_maps = []
    for ci in range(n_cores):
        xT_c = features[ci].reshape(D, n_loc)
        in_maps.append({
            "xT": np.ascontiguousarray(xT_c),
            "xrows": np.ascontiguousarray(xT_c.T),
            "bankT2": bankT2,
            "bank": memory_bank,
            "bsqn": bsqn,
            "eps": np.ascontiguousarray(eps[ci * n_loc:(ci + 1) * n_loc]),
            "w9": w9,
            "iota": iota,
            "cbase": cbase,
            "dw": dw,
        })
    return in_maps


_PROGRAM_CACHE = {}
TRACE = False
TRACE_DIR = None
LAST_EXEC_NS = None


def _get_program():
    key = "full"
    if key not in _PROGRAM_CACHE:
        _PROGRAM_CACHE[key] = build_program()
    return _PROGRAM_CACHE[key]


def kernel(features, memory_bank, influence_weight, distance_weight, eps):
    from concourse.bass_utils import run_bass_kernel_spmd

    nc = _get_program()
    in_maps = make_in_maps(features, memory_bank, influence_weight,
                           distance_weight, eps)
    kw = {}
    if TRACE:
        import tempfile
        d = tempfile.mkdtemp(prefix="knl_trace_", dir=TRACE_DIR)
        globals()["LAST_TRACE_DIR"] = d
        kw = dict(trace=True, tmpdir=d)
    res = run_bass_kernel_spmd(nc, in_maps, core_ids=list(range(N_CORES)), **kw)
    globals()["LAST_EXEC_NS"] = getattr(res, "exec_time_ns", None)
    results = res.results
    n = N_LOC_FULL
    noised = np.concatenate([np.asarray(results[i]["out_noised"])
                             for i in range(N_CORES)], axis=0)
    maps = np.concatenate([np.asarray(results[i]["out_maps"])
                           for i in range(N_CORES)], axis=1)
    noised = noised.reshape(B, H, W, C).transpose(0, 3, 1, 2)
    imap = maps[0].reshape(B, H, W)
    smap = maps[1].reshape(B, H, W)
    return (np.ascontiguousarray(noised).astype(np.float32),
            np.ascontiguousarray(imap).astype(np.float32),
            np.ascontiguousarray(smap).astype(np.float32))
